# revision 1
# baseline (speedup 1.0000x reference)
"""HOG generator kernel for Trainium2, data-parallel over 8 NeuronCores.

Algorithm (per image, validated against the jax reference in numpy):
  - Sobel gx/gy as separable convs: horizontal part on DVE (shifted APs,
    reflect edge cols exact-zero / doubled), vertical part on PE via banded
    113x112 matrices that fold in reflect padding.
  - Orientation binning without atan2: bin boundaries k*pi/9 become sign
    tests of q_k = +-(A - tan_k*B), A = gx^2, B = gx*gy.  Cumulative masked
    magnitudes t_k = magG * [q_k > 0] via ACT Sigmoid(q*1e30 - 40) (exact
    0/1 off the boundary sliver) times magG on DVE.
  - Gaussian weighting folded into the magnitude: row factor via ACT Sqrt
    scale, column factor via one TT multiply with a constant tile.
  - 8x8 pooling: columns via strided tensor_reduce, rows via PE pool matmul.
  - Bin histograms = adjacent differences of the cumulative pools, then
    L2-normalized over the 9 bins.  Device output is (img, 28, 9, 28);
    the final (b, 196, 36) unfold permutation happens on the host.
"""
import math
import sys

import numpy as np

sys.path.insert(0, "/opt/trn_rl_repo")

import concourse.bass as bass
import concourse.bacc as bacc
import concourse.mybir as mybir
from concourse import tile
from concourse.bass_utils import run_bass_kernel_spmd

N_CORES = 8
IMGS_PER_CORE = 16
H = W = 224
NB = 9
F32 = mybir.dt.float32
BF16 = mybir.dt.bfloat16
AF = mybir.ActivationFunctionType
OP = mybir.AluOpType
TANS = [math.tan(k * math.pi / 9.0) for k in range(1, 9)]


def _host_constants(weight_x, gaussian_kernel):
    """Derive the device constant tensors from the module inputs."""
    wx = np.asarray(weight_x, np.float32).reshape(3, 3)
    v_s = wx[:, 0].copy()                      # [1,2,1] vertical smooth
    v_d = wx[0, :].copy()                      # [1,0,-1] -> vertical diff vec
    g2 = np.asarray(gaussian_kernel, np.float64).reshape(16, 16)
    wt = np.sqrt(np.diag(g2)).astype(np.float32)   # g2[i,j] == wt[i]*wt[j]

    def band(chunk, vec):
        m = np.zeros((113, 112), np.float32)
        for i in range(112):
            for d in range(3):
                if chunk == 0:
                    r = i - 1 + d
                    if r == -1:
                        r = 1
                else:
                    r = i + d
                    if r == 113:
                        r = 111
                m[r, i] += vec[d]
        return m

    poolm = np.zeros((112, 14), np.float32)
    for r in range(112):
        poolm[r, r // 8] = 1.0

    blob = np.zeros((113, 689), np.float32)
    blob[:, 0:112] = band(0, v_s)
    blob[:, 112:224] = band(1, v_s)
    blob[:, 224:336] = band(0, v_d)
    blob[:, 336:448] = band(1, v_d)
    blob[0:112, 448:462] = poolm
    blob[:, 462:686] = wt[np.arange(224) % 16][None, :]
    blob[0:112, 686] = wt[np.arange(112) % 16] ** 2
    blob[:, 687] = 0.0
    blob[:, 688] = -40.0
    return {"consts": blob}


def _rep(ap, n, pos=1):
    """Insert a broadcast (step-0) dim of size n into an AP at free pos."""
    import copy
    l = [list(d) for d in ap.ap]
    l.insert(pos, [0, n])
    return bass.AP(ap.tensor, ap.offset, l)


def build_program(n_img=IMGS_PER_CORE):
    assert n_img % 2 == 0
    nc = bacc.Bacc("TRN2", debug=False)
    x_d = nc.dram_tensor("x", [n_img, 224, 224], F32, kind="ExternalInput").ap()
    const_d = nc.dram_tensor("consts", [113, 689], F32, kind="ExternalInput").ap()
    out_d = nc.dram_tensor("out", [n_img, 28, NB, 28], F32, kind="ExternalOutput").ap()
    AX = mybir.AxisListType.X

    with tile.TileContext(nc) as tc:
        with (
            tc.tile_pool(name="const", bufs=1) as cp,
            tc.tile_pool(name="work", bufs=3) as wp,
            tc.tile_pool(name="small", bufs=3) as sp,
            tc.tile_pool(name="psum", bufs=2, space="PSUM") as pp,
            tc.tile_pool(name="psum2", bufs=2, space="PSUM") as pp2,
        ):
            CT = cp.tile([113, 689], F32, tag="CT")
            nc.sync.dma_start(CT[:, :], const_d)
            lhs_s = [CT[:, 0:112], CT[:, 112:224]]
            lhs_d = [CT[:, 224:336], CT[:, 336:448]]
            poolm_ap = CT[0:112, 448:462]
            gc_ap = CT[0:112, 462:686]
            gr2_ap = CT[0:112, 686:687]
            zb = CT[:, 687:688]
            nb40 = CT[:, 688:689]

            pending = [None]

            def flush_norm():
                if pending[0] is None:
                    return
                Hh, ss, i0, ch = pending[0]
                pending[0] = None
                nrm = sp.tile([14, 56], F32, tag="nrm")
                nc.scalar.activation(nrm[:, :], ss[:, :], AF.Sqrt,
                                     bias=zb[0:14, 0:1])
                nc.vector.tensor_scalar_max(nrm[:, :], nrm[:, :], 1e-12)
                inv = sp.tile([14, 56], F32, tag="inv")
                nc.vector.reciprocal(inv[:, :], nrm[:, :])
                OUT = sp.tile([14, NB * 56], F32, tag="OUT")
                hv = Hh[:, :].rearrange("p (i k c) -> p i k c", i=2, k=NB)
                ov = OUT[:, :].rearrange("p (i k c) -> p i k c", i=2, k=NB)
                iv = _rep(inv[:, :].rearrange("p (i c) -> p i c", i=2), NB, pos=2)
                nc.vector.tensor_mul(ov, hv, iv)
                nc.gpsimd.dma_start(
                    out_d[i0:i0 + 2, ch * 14:(ch + 1) * 14, :, :]
                    .rearrange("i r k c -> r i k c"), ov)

            for i0 in range(0, n_img, 2):
                for ch in range(2):
                    r0 = 0 if ch == 0 else 111
                    X = wp.tile([113, 448], F32, tag="X")
                    nc.sync.dma_start(X[:, 0:224], x_d[i0, r0:r0 + 113, :])
                    nc.scalar.dma_start(X[:, 224:448], x_d[i0 + 1, r0:r0 + 113, :])
                    Xv = X[:, :].rearrange("p (i c) -> p i c", i=2)

                    D = wp.tile([113, 448], F32, tag="D")
                    Dv = D[:, :].rearrange("p (i c) -> p i c", i=2)
                    nc.gpsimd.memset(Dv[:, :, 0:1], 0.0)
                    nc.gpsimd.memset(Dv[:, :, 223:224], 0.0)
                    nc.vector.tensor_sub(Dv[:, :, 1:223], Xv[:, :, 0:222],
                                         Xv[:, :, 2:224])

                    S = wp.tile([113, 448], F32, tag="S")
                    Sv = S[:, :].rearrange("p (i c) -> p i c", i=2)
                    nc.vector.scalar_tensor_tensor(
                        Sv[:, :, 1:223], Xv[:, :, 1:223], 2.0, Xv[:, :, 0:222],
                        OP.mult, OP.add)
                    nc.vector.tensor_add(Sv[:, :, 1:223], Sv[:, :, 1:223],
                                         Xv[:, :, 2:224])
                    nc.gpsimd.tensor_add(Sv[:, :, 0:1], Xv[:, :, 0:1], Xv[:, :, 1:2])
                    nc.gpsimd.tensor_scalar_mul(Sv[:, :, 0:1], Sv[:, :, 0:1], 2.0)
                    nc.gpsimd.tensor_add(Sv[:, :, 223:224], Xv[:, :, 222:223],
                                         Xv[:, :, 223:224])
                    nc.gpsimd.tensor_scalar_mul(Sv[:, :, 223:224],
                                                Sv[:, :, 223:224], 2.0)

                    gxp = pp.tile([112, 448], F32, tag="gx")
                    gyp = pp.tile([112, 448], F32, tag="gy")
                    nc.tensor.matmul(gxp[:, :], lhs_s[ch], D[:, :],
                                     start=True, stop=True)
                    nc.tensor.matmul(gyp[:, :], lhs_d[ch], S[:, :],
                                     start=True, stop=True)

                    gys = wp.tile([112, 448], F32, tag="gys")
                    nc.scalar.activation(gys[:, :], gyp[:, :], AF.Copy)
                    A = wp.tile([112, 448], F32, tag="A")
                    nc.scalar.activation(A[:, :], gxp[:, :], AF.Square,
                                         bias=zb[0:112, 0:1])
                    C = wp.tile([112, 448], F32, tag="C")
                    nc.scalar.activation(C[:, :], gyp[:, :], AF.Square,
                                         bias=zb[0:112, 0:1])
                    Bt = wp.tile([112, 448], F32, tag="Bt")
                    nc.vector.tensor_mul(Bt[:, :], gxp[:, :], gys[:, :])
                    S2 = wp.tile([112, 448], F32, tag="S2")
                    nc.gpsimd.tensor_add(S2[:, :], A[:, :], C[:, :])
                    mg = wp.tile([112, 448], F32, tag="mg")
                    nc.scalar.activation(mg[:, :], S2[:, :], AF.Sqrt,
                                         bias=zb[0:112, 0:1], scale=gr2_ap)
                    flush_norm()
                    magG = wp.tile([112, 448], F32, tag="magG")
                    nc.vector.tensor_mul(magG[:, :], mg[:, :],
                                         _rep(gc_ap, 2))
                    magG16 = wp.tile([112, 448], BF16, tag="magG16")
                    nc.vector.tensor_copy(magG16[:, :], magG[:, :])

                    SG = wp.tile([112, 8 * 448], BF16, tag="SG")
                    for k in range(1, NB):
                        tk = TANS[k - 1]
                        q = wp.tile([112, 448], F32, tag="q")
                        eng = nc.vector
                        if k <= 4:
                            eng.scalar_tensor_tensor(
                                q[:, :], Bt[:, :], -tk, A[:, :], OP.mult, OP.add)
                        else:
                            eng.scalar_tensor_tensor(
                                q[:, :], Bt[:, :], tk, A[:, :], OP.mult, OP.subtract)
                        nc.scalar.activation(SG[:, (k - 1) * 448:k * 448],
                                             q[:, :], AF.Sigmoid,
                                             bias=nb40[0:112, 0:1], scale=1e30)
                    nc.vector.tensor_mul(SG[:, :], SG[:, :],
                                         _rep(magG16[:, :], 8))

                    CP = wp.tile([112, NB * 56], F32, tag="CP")
                    cpv = CP[:, :].rearrange("p (i k c) -> p k i c", i=2, k=NB)
                    nc.vector.reduce_sum(
                        cpv[:, 0:1, :, :].rearrange("p k i c -> p (k i) c"),
                        magG[:, :].rearrange("p (i c e) -> p i c e", i=2, e=8),
                        axis=AX)
                    nc.vector.reduce_sum(
                        cpv[:, 1:NB, :, :],
                        SG[:, :].rearrange("p (k i c e) -> p k i c e",
                                           k=8, i=2, e=8),
                        axis=AX)

                    Pp = pp2.tile([14, NB * 56], F32, tag="Pp")
                    nc.tensor.matmul(Pp[:, :], poolm_ap, CP[:, :],
                                     start=True, stop=True)
                    Ps = sp.tile([14, NB * 56], F32, tag="Ps")
                    nc.vector.tensor_copy(Ps[:, :], Pp[:, :])
                    psv = Ps[:, :].rearrange("p (i k c) -> p i k c", i=2, k=NB)
                    Hh = sp.tile([14, NB * 56], F32, tag="Hh")
                    hhv = Hh[:, :].rearrange("p (i k c) -> p i k c", i=2, k=NB)
                    nc.vector.tensor_sub(hhv[:, :, 0:8, :], psv[:, :, 0:8, :],
                                         psv[:, :, 1:9, :])
                    nc.vector.tensor_copy(hhv[:, :, 8, :], psv[:, :, 8, :])
                    sq = sp.tile([14, NB * 56], F32, tag="sq")
                    nc.gpsimd.tensor_mul(sq[:, :], Hh[:, :], Hh[:, :])
                    ss = sp.tile([14, 56], F32, tag="ss")
                    nc.vector.reduce_sum(
                        ss[:, :].rearrange("p (i c) -> p i c", i=2),
                        sq[:, :].rearrange("p (i k c) -> p i c k", i=2, k=NB),
                        axis=AX)
                    pending[0] = (Hh, ss, i0, ch)
            flush_norm()
    nc.compile()
    return nc


def _install_ntff_shim():
    """Provide antenv.axon_hooks (absent in this image) so trace=True works."""
    import sys as _sys
    if "antenv.axon_hooks" in _sys.modules:
        return
    import contextlib
    import ctypes
    import types

    so_path = "/opt/axon/libaxon_pjrt.so"
    lib = ctypes.CDLL(so_path)
    if not hasattr(lib, "axon_start_nrt_profile"):
        hook = None
    else:
        lib.axon_start_nrt_profile.argtypes = [
            ctypes.POINTER(ctypes.c_int64), ctypes.c_size_t]
        lib.axon_start_nrt_profile.restype = ctypes.c_int64
        lib.axon_stop_nrt_profile.argtypes = [ctypes.c_char_p]
        lib.axon_stop_nrt_profile.restype = ctypes.c_int64

        @contextlib.contextmanager
        def hook(output_dir, device_ids):
            import jax
            jax.devices()
            if device_ids:
                ids = (ctypes.c_int64 * len(device_ids))(*device_ids)
                rc = lib.axon_start_nrt_profile(ids, len(device_ids))
            else:
                rc = lib.axon_start_nrt_profile(None, 0)
            if rc != 0:
                raise RuntimeError(f"axon_start_nrt_profile rc={rc}")
            try:
                yield
            finally:
                n = lib.axon_stop_nrt_profile(str(output_dir).encode())
                print(f"profile: {n} file(s) written to {output_dir}",
                      file=sys.stderr)

    mod = types.ModuleType("antenv.axon_hooks")
    mod._hook = hook
    mod.get_axon_ntff_profile_hook = lambda: mod._hook
    mod.set_axon_ntff_profile_hook = lambda h: setattr(mod, "_hook", h)
    _sys.modules["antenv.axon_hooks"] = mod


_prog_cache = {}


def _get_prog(n_img):
    if n_img not in _prog_cache:
        _prog_cache[n_img] = build_program(n_img)
    return _prog_cache[n_img]


def kernel(x, weight_x, weight_y, gaussian_kernel, _trace=False):
    x = np.ascontiguousarray(np.asarray(x, np.float32).reshape(128, 224, 224))
    consts = _host_constants(weight_x, gaussian_kernel)
    nc = _get_prog(IMGS_PER_CORE)
    in_maps = []
    for c in range(N_CORES):
        m = {"x": x[c * IMGS_PER_CORE:(c + 1) * IMGS_PER_CORE]}
        m.update(consts)
        in_maps.append(m)
    if _trace:
        _install_ntff_shim()
    res = run_bass_kernel_spmd(nc, in_maps, core_ids=list(range(N_CORES)),
                               trace=_trace)
    outs = [r["out"] for r in res.results]            # (16, 28, 9, 28) each
    full = np.concatenate(outs, axis=0)               # (128, 28, 9, 28)
    feat = full.transpose(0, 2, 1, 3)                 # (b, 9, 28, 28)
    feat = feat.transpose(0, 2, 3, 1)                 # (b, 28, 28, 9)
    feat = feat.reshape(128, 14, 2, 14, 2, NB)
    feat = feat.transpose(0, 1, 3, 5, 2, 4).reshape(128, 196, NB * 4)
    if _trace:
        return np.ascontiguousarray(feat), res
    return np.ascontiguousarray(feat)

